# revision 21
# baseline (speedup 1.0000x reference)
"""Trainium2 Bass kernel for nn_EntailmentSelfAttention (8-core data parallel).

Problem (per batch element n, sentence s):
  q/k/v head projections (shared per-head weights), energy = q @ k.T per head,
  query-position masking, softmax over the QUERY axis, out = attn @ v,
  fc_out: out @ Wo.T + bo.

Mapping (one (n) per NeuronCore; S=2 sentences iterated inside):
  - Tensors kept "transposed" on-chip: head-dim/embed-dim on partitions,
    sequence on the free axis, so the softmax (over queries) reduces along the
    free axis.
  - The V projection is folded into fc_out on the host:
      out = concat_h((attn_h @ xv_h) @ Wv.T) @ Wo.T = concat_h(attn_h @ xv_h) @ Wcomb
  - The query mask enters the energy matmul as a 65th contraction row:
      kT_ext row64 = 1, qT_ext row64 = -3000 * (1 - mask_q); after the 1/sqrt(L)
      softmax scale the exp underflows to 0 exactly, matching -1e20 semantics.
  - The K projection is folded into the q-side projection on the host:
      energy^T = xk^T (Wk^T Wq) xq.
  - Masked query positions are dropped on the host (compaction to QP columns);
    QP is padded to a multiple of 16.  Their output rows are exactly the fc
    bias; the bias for surviving rows is also added on the host.
  - Energy PSUM tiles are 2-bank pairs [128, 2, 512] so the exp activation,
    the PSUM->SBUF copies and the fc output copies each cover two heads /
    two fc column-tiles per instruction.
  - Softmax denominators: a tunable subset of (s,g,c) units uses the Exp
    activation's accum_out (scalar engine); the rest use a merged DVE
    tensor_reduce over the bf16 attention pair-tile.
  - The 1/rowsum normalization is folded into a per-k-row scale of xv before
    the attn @ xv matmul (tensor_tensor, optionally on gpsimd).
"""

import math

import numpy as np

import concourse.bass as bass
import concourse.tile as tile
from concourse import bacc, mybir
from concourse import bass_utils

# problem shapes (hardcoded per the harness contract)
N, S, L, E, H = 8, 2, 512, 1024, 16
D = E // H  # 64
DX = D + 1  # extended head dim (projection + mask/ones row)
P = 128
NCORES = 8
LC = L // P  # 4 l-chunks
GH = 4  # heads per group
NG = H // GH  # 4 groups
BMASK = 3000.0
QP_MIN = 256
SCALE = 1.0 / math.sqrt(float(L))

F32 = mybir.dt.float32
BF16 = mybir.dt.bfloat16

# rowsum strategy per (s, g, c) unit index (0..31):
#   "acc"    - per-head exp with scalar-engine fused accumulate
#   "reduce" - merged pair exp + DVE tensor_reduce per pair
#   "ts"     - merged pair exp + per-head DVE tensor_scalar w/ accum_out
#              (single-src op: eligible for DVE 2x/4x perf modes)
N_ACC = 6
ACC_UNITS = frozenset(
    u for u in range(32) if (u * N_ACC) // 32 != ((u + 1) * N_ACC) // 32
)
DVE_ROWSUM = "reduce"
# xvs scaling engine: "gpsimd" offloads the tensor_tensor to the Pool engine
XVS_ENGINE = "gpsimd"


def build_kernel_body(tc, outs, ins, QP):
    nc = tc.nc

    def _c(ap):
        # sim path: run_kernel declares DRAM as plain fp32; view as bf16
        return ap if ap.dtype == BF16 else ap.bitcast(BF16)

    xq, xk, xv = _c(ins["xq"]), _c(ins["xk"]), _c(ins["xv"])
    wq, wcomb = _c(ins["wq"]), _c(ins["wcomb"])
    outT = outs["outT"]

    xvs_eng = nc.gpsimd if XVS_ENGINE == "gpsimd" else nc.vector

    import contextlib

    with contextlib.ExitStack() as ctx:
        ek = ctx.enter_context
        consts = ek(tc.tile_pool(name="consts", bufs=1))
        xqpool = ek(tc.tile_pool(name="xqp", bufs=5))
        xkpool = ek(tc.tile_pool(name="xkp", bufs=5))
        xvpool = ek(tc.tile_pool(name="xvp", bufs=2))
        qepool = ek(tc.tile_pool(name="qe", bufs=2))
        atpool = ek(tc.tile_pool(name="attn", bufs=4))
        scrpool = ek(tc.tile_pool(name="scrap", bufs=2))
        sumpool = ek(tc.tile_pool(name="sums", bufs=8))
        xvspool = ek(tc.tile_pool(name="xvs", bufs=4))
        ztpool = ek(tc.tile_pool(name="zt", bufs=2))
        otpool = ek(tc.tile_pool(name="out", bufs=3))
        pp_e = ek(tc.tile_pool(name="pp_e", bufs=3, space="PSUM"))
        pp_z = ek(tc.tile_pool(name="pp_z", bufs=1, space="PSUM"))

        wq_sb = consts.tile([DX, DX], BF16, tag="wq")
        wcomb_sb = consts.tile([P, E // P, E], BF16, tag="wcomb")

        # startup: group-0 inputs fan out across engine queues so the DGE
        # configs don't serialize; everything else trickles in on sync/gpsimd.
        xq_gs = {}
        xk_gs = {}
        xv_sbs = {}

        def load_group(s, g, qeng, keng):
            t = xqpool.tile([DX, GH, QP], BF16, tag="xq_g", name=f"xq_{s}_{g}")
            qeng.dma_start(t[:], xq[s, g])
            xq_gs[(s, g)] = t
            t = xkpool.tile([DX, GH, L], BF16, tag="xk_g", name=f"xk_{s}_{g}")
            keng.dma_start(t[:], xk[s, g])
            xk_gs[(s, g)] = t

        nc.scalar.dma_start(wq_sb[:], wq[:])
        load_group(0, 0, nc.sync, nc.gpsimd)
        xv_sbs[0] = xvpool.tile([P, LC, E], BF16, tag="xv", name="xv_0")
        nc.gpsimd.dma_start(xv_sbs[0][:, 0, :], xv[0, 0])
        for c in range(1, LC):
            nc.gpsimd.dma_start(xv_sbs[0][:, c, :], xv[0, c])
        for g in range(1, NG):
            load_group(0, g, nc.sync, nc.sync)
        nc.gpsimd.dma_start(
            wcomb_sb[:], wcomb.rearrange("(eo p) j -> p eo j", p=P))

        ZTs = {}

        def emit_fc(s, jp):
            # fc_out: two column-tiles per PSUM pair-slot; bias added on host
            fp = pp_e.tile([P, 2, 512], F32, tag="ep", name=f"fp_{s}_{jp}")
            for j in range(2):
                jt = 2 * jp + j
                for eo in range(E // P):
                    nc.tensor.matmul(
                        fp[:, j, :QP],
                        wcomb_sb[:, eo, jt * P:(jt + 1) * P],
                        ZTs[s][:, eo, :],
                        start=(eo == 0),
                        stop=(eo == E // P - 1))
            ot = otpool.tile([P, 2, QP], BF16, tag="ot", name=f"ot_{s}_{jp}")
            nc.vector.tensor_copy(ot[:], fp[:, :, :QP])
            nc.sync.dma_start(outT[s, :, jp], ot[:])

        def emit_av(zp, xvs, at, c):
            for i in range(GH):
                nc.tensor.matmul(
                    zp[(i % 2) * D:(i % 2 + 1) * D, i // 2, :QP],
                    xvs[:, i],
                    at[:, i, :],
                    start=(c == 0),
                    stop=(c == LC - 1),
                    skip_group_check=True)

        def emit_qproj(s, g):
            # q projections: two heads per PSUM pair-slot; the two merged
            # copies split across scalar/vector so neither queue head-blocks
            xq_g = xq_gs[(s, g)]
            qe = qepool.tile([DX, GH, QP], BF16, tag="qe", name=f"qe_{s}_{g}")
            for p2 in range(GH // 2):
                pq = pp_e.tile([P, 2, 512], F32, tag="ep", name="pq")
                for j in range(2):
                    nc.tensor.matmul(
                        pq[:DX, j, :QP], wq_sb[:], xq_g[:, 2 * p2 + j, :],
                        start=True, stop=True)
                if p2 == 0:
                    nc.scalar.copy(qe[:, :2, :], pq[:DX, :, :QP])
                else:
                    nc.vector.tensor_copy(qe[:, 2:, :], pq[:DX, :, :QP])
            return qe

        qe_next = None
        for s in range(S):
            xv_sb = xv_sbs[s]
            ZTs[s] = ztpool.tile([P, E // P, QP], BF16, tag="zt", name=f"zt_{s}")
            for g in range(NG):
                xk_g = xk_gs[(s, g)]
                # stream in the next sentence's inputs two groups ahead
                if s == 0 and g >= 1 and g <= 2:
                    load_group(1, 2 * (g - 1), nc.sync, nc.sync)
                    load_group(1, 2 * (g - 1) + 1, nc.sync, nc.sync)
                if s == 0 and g == 3:
                    xv_sbs[1] = xvpool.tile([P, LC, E], BF16, tag="xv",
                                            name="xv_1")
                    for c in range(LC):
                        nc.sync.dma_start(xv_sbs[1][:, c, :], xv[1, c])

                qe = emit_qproj(s, g) if qe_next is None else qe_next
                qe_next = None

                zp = pp_z.tile([P, 2, 512], F32, tag="zp", name=f"zp_{s}_{g}")
                pend_av = None  # (xvs, at, c) for software-pipelined attn@v
                for c in range(LC):
                    unit = (s * NG + g) * LC + c
                    acc_mode = unit in ACC_UNITS
                    rsum = sumpool.tile([P, GH], F32, tag="rsum")
                    at = atpool.tile([P, GH, QP], BF16, tag="at", name="at")
                    for p2 in range(GH // 2):
                        ep = pp_e.tile([P, 2, 512], F32, tag="ep", name="ep")
                        for j in range(2):
                            i = 2 * p2 + j
                            nc.tensor.matmul(
                                ep[:, j, :QP],
                                xk_g[:, i, c * P:(c + 1) * P],
                                qe[:, i, :],
                                start=True, stop=True)
                        if acc_mode:
                            for j in range(2):
                                i = 2 * p2 + j
                                nc.scalar.activation(
                                    at[:, i, :], ep[:, j, :QP],
                                    mybir.ActivationFunctionType.Exp,
                                    scale=SCALE,
                                    accum_out=rsum[:, i:i + 1])
                        else:
                            nc.scalar.activation(
                                at[:, 2 * p2:2 * p2 + 2, :], ep[:, :, :QP],
                                mybir.ActivationFunctionType.Exp,
                                scale=SCALE)
                            nc.vector.tensor_reduce(
                                rsum[:, 2 * p2:2 * p2 + 2],
                                at[:, 2 * p2:2 * p2 + 2, :],
                                axis=mybir.AxisListType.X,
                                op=mybir.AluOpType.add)
                    if c == LC - 2:
                        # hoist the next group's q-projection: emitted early
                        # so its copies' deps are met when they reach the
                        # scalar/vector queue heads
                        if g < NG - 1:
                            qe_next = emit_qproj(s, g + 1)
                        elif s < S - 1:
                            qe_next = emit_qproj(s + 1, 0)
                    recip = sumpool.tile([P, GH], F32, tag="recip")
                    nc.vector.reciprocal(recip[:], rsum[:])
                    # xvs[p, i, d] = xv[p, c, (g*GH+i)*64+d] * recip[p, i]
                    xvs = xvspool.tile([P, GH, D], BF16, tag="xvs")
                    xvs_eng.tensor_tensor(
                        xvs[:],
                        xv_sb[:, c, g * GH * D:(g + 1) * GH * D].rearrange(
                            "p (h d) -> p h d", d=D),
                        recip[:, :, None].to_broadcast((P, GH, D)),
                        mybir.AluOpType.mult)
                    if pend_av is not None:
                        emit_av(zp, *pend_av)
                    pend_av = (xvs, at, c)
                emit_av(zp, *pend_av)
                nc.vector.tensor_copy(ZTs[s][:, 2 * g:2 * g + 2, :],
                                      zp[:, :, :QP])
                # interleave the previous sentence's fc into this attention
                if s == 1:
                    emit_fc(0, g)
            if s == 1:
                for jp in range(E // P // 2):
                    emit_fc(1, jp)


def host_prepare(values, keys, query, mask, Wv, Wk, Wq, Wo, bo):
    """Host-side sharding + layout + query compaction.

    Returns (in_maps, QP, order, cnt, bo_np). Masked query positions are
    dropped entirely (their output is just bo); the surviving queries are
    compacted to the front and padded to QP columns. Pad columns carry a
    -BMASK bias row so their exp is exactly 0 (excluded from denominators).
    """
    values = np.ascontiguousarray(np.asarray(values, dtype=np.float32))
    keys = np.asarray(keys, dtype=np.float32)
    query = np.asarray(query, dtype=np.float32)
    mask = np.asarray(mask)
    Wv = np.asarray(Wv, dtype=np.float32)
    Wk = np.asarray(Wk, dtype=np.float32)
    Wq = np.asarray(Wq, dtype=np.float32)
    Wo = np.asarray(Wo, dtype=np.float32)
    bo_np = np.ascontiguousarray(np.asarray(bo, dtype=np.float32))

    keep = mask[:, :, :, 0] != 0  # (N, S, L) True = query position survives
    cnt = keep.sum(-1)  # (N, S)
    QP = int(np.ceil(max(int(cnt.max()), 1) / 16) * 16)
    QP = max(QP, QP_MIN)
    QP = min(QP, L)
    # stable partition: surviving query indices first
    order = np.argsort(~keep, axis=-1, kind="stable")  # (N, S, L)

    qT = query.transpose(0, 1, 3, 2).reshape(N, S, H, D, L)
    kT = keys.transpose(0, 1, 3, 2).reshape(N, S, H, D, L)

    # gather+pad queries: (N, S, H, D, QP)
    gidx = order[:, :, :QP]  # (N, S, QP)
    qTc = np.take_along_axis(
        qT, gidx[:, :, None, None, :].repeat(H, 2).repeat(D, 3), axis=4)
    pad = np.arange(QP)[None, None, :] >= cnt[:, :, None]  # (N, S, QP)
    qTc[pad[:, :, None, None, :].repeat(H, 2).repeat(D, 3)] = 0.0
    qb_row = np.where(pad, np.float32(-BMASK), np.float32(0.0)).astype(np.float32)
    xq = np.concatenate([qTc, qb_row[:, :, None, None, :].repeat(H, 2)], axis=3)
    # (N,S,H,DX,QP) -> (N,S,NG,DX,GH,QP) so each group is one contiguous DMA
    xq = np.ascontiguousarray(
        xq.reshape(N, S, NG, GH, DX, QP).transpose(0, 1, 2, 4, 3, 5))

    ones_row = np.ones((N, S, H, 1, L), np.float32)
    xk = np.concatenate([kT, ones_row], axis=3)
    xk = np.ascontiguousarray(
        xk.reshape(N, S, NG, GH, DX, L).transpose(0, 1, 2, 4, 3, 5))

    # fused projection: energyT = xk^T (Wk^T Wq) xq -> yq = (Wk^T Wq) @ xqT,
    # lhsT[dj, di] = (Wk^T Wq)[di, dj] = (Wq^T Wk)[dj, di]
    wq_ext = np.zeros((DX, DX), np.float32)
    wq_ext[:D, :D] = Wq.T @ Wk
    wq_ext[D, D] = 1.0

    wcomb = np.zeros((E, E), np.float32)
    for h in range(H):
        wcomb[h * D:(h + 1) * D, :] = Wv.T @ Wo[:, h * D:(h + 1) * D].T
    wcomb = np.ascontiguousarray(wcomb)

    import ml_dtypes
    bf = ml_dtypes.bfloat16
    # values as (N, S, LC, P, E): per-(s, l-chunk) DMA granularity
    values_bf = np.ascontiguousarray(
        values.reshape(N, S, LC, P, E).astype(bf))
    xq = np.ascontiguousarray(xq.astype(bf))
    xk = np.ascontiguousarray(xk.astype(bf))
    wq_ext = wq_ext.astype(bf)
    wcomb = np.ascontiguousarray(wcomb.astype(bf))
    shared = {"wq": wq_ext, "wcomb": wcomb}
    in_maps = []
    for n in range(NCORES):
        m = {"xq": xq[n], "xk": xk[n], "xv": values_bf[n]}
        m.update(shared)
        in_maps.append(m)
    return in_maps, QP, order, cnt, bo_np


_NC_CACHE = {}


def _get_program(QP):
    nc = _NC_CACHE.get(QP)
    if nc is not None:
        return nc
    nc = bacc.Bacc("TRN2", target_bir_lowering=False, debug=False,
                   num_devices=NCORES)
    ins = {
        "xq": nc.dram_tensor("xq", (S, NG, DX, GH, QP), BF16, kind="ExternalInput").ap(),
        "xk": nc.dram_tensor("xk", (S, NG, DX, GH, L), BF16, kind="ExternalInput").ap(),
        "xv": nc.dram_tensor("xv", (S, LC, P, E), BF16, kind="ExternalInput").ap(),
        "wq": nc.dram_tensor("wq", (DX, DX), BF16, kind="ExternalInput").ap(),
        "wcomb": nc.dram_tensor("wcomb", (E, E), BF16, kind="ExternalInput").ap(),
    }
    outs = {
        "outT": nc.dram_tensor("outT", (S, P, E // P // 2, 2, QP), BF16,
                               kind="ExternalOutput").ap(),
    }
    with tile.TileContext(nc) as tc:
        build_kernel_body(tc, outs, ins, QP)
    nc.compile()
    _NC_CACHE[QP] = nc
    return nc


def run(inputs: dict, trace: bool = False):
    """Run on 8 cores; returns (full_output, BassKernelResults)."""
    in_maps, QP, order, cnt, bo_np = host_prepare(**inputs)
    nc = _get_program(QP)
    res = bass_utils.run_bass_kernel_spmd(
        nc, in_maps, core_ids=list(range(NCORES)), trace=trace,
    )
    out = np.empty((N, S, L, E), np.float32)
    out[:] = bo_np  # masked query rows: attention output is 0, fc adds bo
    for n in range(NCORES):
        # (S, P, 4, 2, QP) -> (S, E, QP): e = (jp*2 + j)*128 + p
        oT = np.asarray(res.results[n]["outT"]).astype(np.float32)
        oT = oT.transpose(0, 2, 3, 1, 4).reshape(S, E, QP)
        for s in range(S):
            c = int(cnt[n, s])
            if c:
                out[n, s, order[n, s, :c], :] = oT[s, :, :c].T + bo_np
    return out, res


def kernel(**inputs) -> np.ndarray:
    out, _ = run(inputs, trace=False)
    return out


# revision 22
# speedup vs baseline: 1.1017x; 1.1017x over previous
"""Trainium2 Bass kernel for nn_EntailmentSelfAttention (8-core data parallel).

Problem (per batch element n, sentence s):
  q/k/v head projections (shared per-head weights), energy = q @ k.T per head,
  query-position masking, softmax over the QUERY axis, out = attn @ v,
  fc_out: out @ Wo.T + bo.

Mapping (one (n) per NeuronCore; S=2 sentences iterated inside):
  - Tensors kept "transposed" on-chip: head-dim/embed-dim on partitions,
    sequence on the free axis, so the softmax (over queries) reduces along the
    free axis.
  - The V projection is folded into fc_out on the host:
      out = concat_h((attn_h @ xv_h) @ Wv.T) @ Wo.T = concat_h(attn_h @ xv_h) @ Wcomb
  - The query mask enters the energy matmul as a 65th contraction row:
      kT_ext row64 = 1, qT_ext row64 = -3000 * (1 - mask_q); after the 1/sqrt(L)
      softmax scale the exp underflows to 0 exactly, matching -1e20 semantics.
  - The K projection is folded into the q-side projection on the host:
      energy^T = xk^T (Wk^T Wq) xq.
  - Masked query positions are dropped on the host (compaction to QP columns);
    QP is padded to a multiple of 16.  Their output rows are exactly the fc
    bias; the bias for surviving rows is also added on the host.
  - Energy PSUM tiles are 2-bank pairs [128, 2, 512] so the exp activation,
    the PSUM->SBUF copies and the fc output copies each cover two heads /
    two fc column-tiles per instruction.
  - Softmax denominators: a tunable subset of (s,g,c) units uses the Exp
    activation's accum_out (scalar engine); the rest use a merged DVE
    tensor_reduce over the bf16 attention pair-tile.
  - The 1/rowsum normalization is folded into a per-k-row scale of xv before
    the attn @ xv matmul (tensor_tensor, optionally on gpsimd).
"""

import math

import numpy as np

import concourse.bass as bass
import concourse.tile as tile
from concourse import bacc, mybir
from concourse import bass_utils

# problem shapes (hardcoded per the harness contract)
N, S, L, E, H = 8, 2, 512, 1024, 16
D = E // H  # 64
DX = D + 1  # extended head dim (projection + mask/ones row)
P = 128
NCORES = 8
LC = L // P  # 4 l-chunks
GH = 4  # heads per group
NG = H // GH  # 4 groups
BMASK = 3000.0
QP_MIN = 256
SCALE = 1.0 / math.sqrt(float(L))

F32 = mybir.dt.float32
BF16 = mybir.dt.bfloat16

# rowsum strategy per (s, g, c) unit index (0..31):
#   "acc"    - per-head exp with scalar-engine fused accumulate
#   "reduce" - merged pair exp + DVE tensor_reduce per pair
#   "ts"     - merged pair exp + per-head DVE tensor_scalar w/ accum_out
#              (single-src op: eligible for DVE 2x/4x perf modes)
N_ACC = 0
ACC_UNITS = frozenset(
    u for u in range(32) if (u * N_ACC) // 32 != ((u + 1) * N_ACC) // 32
)
DVE_ROWSUM = "reduce"
# xvs scaling engine: "gpsimd" offloads the tensor_tensor to the Pool engine
XVS_ENGINE = "gpsimd"


def build_kernel_body(tc, outs, ins, QP):
    nc = tc.nc

    def _c(ap):
        # sim path: run_kernel declares DRAM as plain fp32; view as bf16
        return ap if ap.dtype == BF16 else ap.bitcast(BF16)

    xq, xk, xv = _c(ins["xq"]), _c(ins["xk"]), _c(ins["xv"])
    wq, wcomb = _c(ins["wq"]), _c(ins["wcomb"])
    outT = outs["outT"]

    xvs_eng = nc.gpsimd if XVS_ENGINE == "gpsimd" else nc.vector

    import contextlib

    with contextlib.ExitStack() as ctx:
        ek = ctx.enter_context
        consts = ek(tc.tile_pool(name="consts", bufs=1))
        xqpool = ek(tc.tile_pool(name="xqp", bufs=5))
        xkpool = ek(tc.tile_pool(name="xkp", bufs=5))
        xvpool = ek(tc.tile_pool(name="xvp", bufs=2))
        qepool = ek(tc.tile_pool(name="qe", bufs=2))
        atpool = ek(tc.tile_pool(name="attn", bufs=4))
        scrpool = ek(tc.tile_pool(name="scrap", bufs=2))
        sumpool = ek(tc.tile_pool(name="sums", bufs=8))
        xvspool = ek(tc.tile_pool(name="xvs", bufs=4))
        ztpool = ek(tc.tile_pool(name="zt", bufs=2))
        otpool = ek(tc.tile_pool(name="out", bufs=3))
        pp_e = ek(tc.tile_pool(name="pp_e", bufs=3, space="PSUM"))
        pp_z = ek(tc.tile_pool(name="pp_z", bufs=1, space="PSUM"))

        wq_sb = consts.tile([DX, DX], BF16, tag="wq")
        wcomb_sb = consts.tile([P, E // P, E], BF16, tag="wcomb")

        # startup: group-0 inputs fan out across engine queues so the DGE
        # configs don't serialize; everything else trickles in on sync/gpsimd.
        xq_gs = {}
        xk_gs = {}
        xv_sbs = {}

        def load_group(s, g, qeng, keng):
            t = xqpool.tile([DX, GH, QP], BF16, tag="xq_g", name=f"xq_{s}_{g}")
            qeng.dma_start(t[:], xq[s, g])
            xq_gs[(s, g)] = t
            t = xkpool.tile([DX, GH, L], BF16, tag="xk_g", name=f"xk_{s}_{g}")
            keng.dma_start(t[:], xk[s, g])
            xk_gs[(s, g)] = t

        nc.scalar.dma_start(wq_sb[:], wq[:])
        load_group(0, 0, nc.sync, nc.gpsimd)
        xv_sbs[0] = xvpool.tile([P, LC, E], BF16, tag="xv", name="xv_0")
        nc.gpsimd.dma_start(xv_sbs[0][:, 0, :], xv[0, 0])
        for c in range(1, LC):
            nc.gpsimd.dma_start(xv_sbs[0][:, c, :], xv[0, c])
        for g in range(1, NG):
            load_group(0, g, nc.sync, nc.sync)
        nc.gpsimd.dma_start(
            wcomb_sb[:], wcomb.rearrange("(eo p) j -> p eo j", p=P))

        ZTs = {}

        def emit_fc(s, jp):
            # fc_out: two column-tiles per PSUM pair-slot; bias added on host
            fp = pp_e.tile([P, 2, 512], F32, tag="ep", name=f"fp_{s}_{jp}")
            for j in range(2):
                jt = 2 * jp + j
                for eo in range(E // P):
                    nc.tensor.matmul(
                        fp[:, j, :QP],
                        wcomb_sb[:, eo, jt * P:(jt + 1) * P],
                        ZTs[s][:, eo, :],
                        start=(eo == 0),
                        stop=(eo == E // P - 1))
            ot = otpool.tile([P, 2, QP], BF16, tag="ot", name=f"ot_{s}_{jp}")
            nc.scalar.copy(ot[:], fp[:, :, :QP])
            nc.sync.dma_start(outT[s, :, jp], ot[:])

        def emit_av(zp, xvs, at, c):
            for i in range(GH):
                nc.tensor.matmul(
                    zp[(i % 2) * D:(i % 2 + 1) * D, i // 2, :QP],
                    xvs[:, i],
                    at[:, i, :],
                    start=(c == 0),
                    stop=(c == LC - 1),
                    skip_group_check=True)

        def emit_qproj(s, g):
            # q projections: two heads per PSUM pair-slot; the two merged
            # copies split across scalar/vector so neither queue head-blocks
            xq_g = xq_gs[(s, g)]
            qe = qepool.tile([DX, GH, QP], BF16, tag="qe", name=f"qe_{s}_{g}")
            for p2 in range(GH // 2):
                pq = pp_e.tile([P, 2, 512], F32, tag="ep", name="pq")
                for j in range(2):
                    nc.tensor.matmul(
                        pq[:DX, j, :QP], wq_sb[:], xq_g[:, 2 * p2 + j, :],
                        start=True, stop=True)
                if p2 == 0:
                    nc.scalar.copy(qe[:, :2, :], pq[:DX, :, :QP])
                else:
                    nc.vector.tensor_copy(qe[:, 2:, :], pq[:DX, :, :QP])
            return qe

        qe_next = None
        for s in range(S):
            xv_sb = xv_sbs[s]
            ZTs[s] = ztpool.tile([P, E // P, QP], BF16, tag="zt", name=f"zt_{s}")
            for g in range(NG):
                xk_g = xk_gs[(s, g)]
                # stream in the next sentence's inputs two groups ahead
                if s == 0 and g >= 1 and g <= 2:
                    load_group(1, 2 * (g - 1), nc.sync, nc.sync)
                    load_group(1, 2 * (g - 1) + 1, nc.sync, nc.sync)
                if s == 0 and g == 3:
                    xv_sbs[1] = xvpool.tile([P, LC, E], BF16, tag="xv",
                                            name="xv_1")
                    for c in range(LC):
                        nc.sync.dma_start(xv_sbs[1][:, c, :], xv[1, c])

                qe = emit_qproj(s, g) if qe_next is None else qe_next
                qe_next = None

                zp = pp_z.tile([P, 2, 512], F32, tag="zp", name=f"zp_{s}_{g}")
                pend_av = None  # (xvs, at, c) for software-pipelined attn@v
                for c in range(LC):
                    unit = (s * NG + g) * LC + c
                    acc_mode = unit in ACC_UNITS
                    rsum = sumpool.tile([P, GH], F32, tag="rsum")
                    at = atpool.tile([P, GH, QP], BF16, tag="at", name="at")
                    for p2 in range(GH // 2):
                        ep = pp_e.tile([P, 2, 512], F32, tag="ep", name="ep")
                        for j in range(2):
                            i = 2 * p2 + j
                            nc.tensor.matmul(
                                ep[:, j, :QP],
                                xk_g[:, i, c * P:(c + 1) * P],
                                qe[:, i, :],
                                start=True, stop=True)
                        if acc_mode:
                            for j in range(2):
                                i = 2 * p2 + j
                                nc.scalar.activation(
                                    at[:, i, :], ep[:, j, :QP],
                                    mybir.ActivationFunctionType.Exp,
                                    scale=SCALE,
                                    accum_out=rsum[:, i:i + 1])
                        else:
                            nc.scalar.activation(
                                at[:, 2 * p2:2 * p2 + 2, :], ep[:, :, :QP],
                                mybir.ActivationFunctionType.Exp,
                                scale=SCALE)
                            nc.vector.tensor_reduce(
                                rsum[:, 2 * p2:2 * p2 + 2],
                                at[:, 2 * p2:2 * p2 + 2, :],
                                axis=mybir.AxisListType.X,
                                op=mybir.AluOpType.add)
                    if c == LC - 2:
                        # hoist the next group's q-projection: emitted early
                        # so its copies' deps are met when they reach the
                        # scalar/vector queue heads
                        if g < NG - 1:
                            qe_next = emit_qproj(s, g + 1)
                        elif s < S - 1:
                            qe_next = emit_qproj(s + 1, 0)
                    recip = sumpool.tile([P, GH], F32, tag="recip")
                    nc.vector.reciprocal(recip[:], rsum[:])
                    # xvs[p, i, d] = xv[p, c, (g*GH+i)*64+d] * recip[p, i]
                    xvs = xvspool.tile([P, GH, D], BF16, tag="xvs")
                    xvs_eng.tensor_tensor(
                        xvs[:],
                        xv_sb[:, c, g * GH * D:(g + 1) * GH * D].rearrange(
                            "p (h d) -> p h d", d=D),
                        recip[:, :, None].to_broadcast((P, GH, D)),
                        mybir.AluOpType.mult)
                    if pend_av is not None:
                        emit_av(zp, *pend_av)
                    pend_av = (xvs, at, c)
                emit_av(zp, *pend_av)
                nc.vector.tensor_copy(ZTs[s][:, 2 * g:2 * g + 2, :],
                                      zp[:, :, :QP])
                # interleave the previous sentence's fc into this attention
                if s == 1:
                    emit_fc(0, g)
            if s == 1:
                for jp in range(E // P // 2):
                    emit_fc(1, jp)


def host_prepare(values, keys, query, mask, Wv, Wk, Wq, Wo, bo):
    """Host-side sharding + layout + query compaction.

    Returns (in_maps, QP, order, cnt, bo_np). Masked query positions are
    dropped entirely (their output is just bo); the surviving queries are
    compacted to the front and padded to QP columns. Pad columns carry a
    -BMASK bias row so their exp is exactly 0 (excluded from denominators).
    """
    values = np.ascontiguousarray(np.asarray(values, dtype=np.float32))
    keys = np.asarray(keys, dtype=np.float32)
    query = np.asarray(query, dtype=np.float32)
    mask = np.asarray(mask)
    Wv = np.asarray(Wv, dtype=np.float32)
    Wk = np.asarray(Wk, dtype=np.float32)
    Wq = np.asarray(Wq, dtype=np.float32)
    Wo = np.asarray(Wo, dtype=np.float32)
    bo_np = np.ascontiguousarray(np.asarray(bo, dtype=np.float32))

    keep = mask[:, :, :, 0] != 0  # (N, S, L) True = query position survives
    cnt = keep.sum(-1)  # (N, S)
    QP = int(np.ceil(max(int(cnt.max()), 1) / 16) * 16)
    QP = max(QP, QP_MIN)
    QP = min(QP, L)
    # stable partition: surviving query indices first
    order = np.argsort(~keep, axis=-1, kind="stable")  # (N, S, L)

    qT = query.transpose(0, 1, 3, 2).reshape(N, S, H, D, L)
    kT = keys.transpose(0, 1, 3, 2).reshape(N, S, H, D, L)

    # gather+pad queries: (N, S, H, D, QP)
    gidx = order[:, :, :QP]  # (N, S, QP)
    qTc = np.take_along_axis(
        qT, gidx[:, :, None, None, :].repeat(H, 2).repeat(D, 3), axis=4)
    pad = np.arange(QP)[None, None, :] >= cnt[:, :, None]  # (N, S, QP)
    qTc[pad[:, :, None, None, :].repeat(H, 2).repeat(D, 3)] = 0.0
    qb_row = np.where(pad, np.float32(-BMASK), np.float32(0.0)).astype(np.float32)
    xq = np.concatenate([qTc, qb_row[:, :, None, None, :].repeat(H, 2)], axis=3)
    # (N,S,H,DX,QP) -> (N,S,NG,DX,GH,QP) so each group is one contiguous DMA
    xq = np.ascontiguousarray(
        xq.reshape(N, S, NG, GH, DX, QP).transpose(0, 1, 2, 4, 3, 5))

    ones_row = np.ones((N, S, H, 1, L), np.float32)
    xk = np.concatenate([kT, ones_row], axis=3)
    xk = np.ascontiguousarray(
        xk.reshape(N, S, NG, GH, DX, L).transpose(0, 1, 2, 4, 3, 5))

    # fused projection: energyT = xk^T (Wk^T Wq) xq -> yq = (Wk^T Wq) @ xqT,
    # lhsT[dj, di] = (Wk^T Wq)[di, dj] = (Wq^T Wk)[dj, di]
    wq_ext = np.zeros((DX, DX), np.float32)
    wq_ext[:D, :D] = Wq.T @ Wk
    wq_ext[D, D] = 1.0

    wcomb = np.zeros((E, E), np.float32)
    for h in range(H):
        wcomb[h * D:(h + 1) * D, :] = Wv.T @ Wo[:, h * D:(h + 1) * D].T
    wcomb = np.ascontiguousarray(wcomb)

    import ml_dtypes
    bf = ml_dtypes.bfloat16
    # values as (N, S, LC, P, E): per-(s, l-chunk) DMA granularity
    values_bf = np.ascontiguousarray(
        values.reshape(N, S, LC, P, E).astype(bf))
    xq = np.ascontiguousarray(xq.astype(bf))
    xk = np.ascontiguousarray(xk.astype(bf))
    wq_ext = wq_ext.astype(bf)
    wcomb = np.ascontiguousarray(wcomb.astype(bf))
    shared = {"wq": wq_ext, "wcomb": wcomb}
    in_maps = []
    for n in range(NCORES):
        m = {"xq": xq[n], "xk": xk[n], "xv": values_bf[n]}
        m.update(shared)
        in_maps.append(m)
    return in_maps, QP, order, cnt, bo_np


_NC_CACHE = {}


def _get_program(QP):
    nc = _NC_CACHE.get(QP)
    if nc is not None:
        return nc
    nc = bacc.Bacc("TRN2", target_bir_lowering=False, debug=False,
                   num_devices=NCORES)
    ins = {
        "xq": nc.dram_tensor("xq", (S, NG, DX, GH, QP), BF16, kind="ExternalInput").ap(),
        "xk": nc.dram_tensor("xk", (S, NG, DX, GH, L), BF16, kind="ExternalInput").ap(),
        "xv": nc.dram_tensor("xv", (S, LC, P, E), BF16, kind="ExternalInput").ap(),
        "wq": nc.dram_tensor("wq", (DX, DX), BF16, kind="ExternalInput").ap(),
        "wcomb": nc.dram_tensor("wcomb", (E, E), BF16, kind="ExternalInput").ap(),
    }
    outs = {
        "outT": nc.dram_tensor("outT", (S, P, E // P // 2, 2, QP), BF16,
                               kind="ExternalOutput").ap(),
    }
    with tile.TileContext(nc) as tc:
        build_kernel_body(tc, outs, ins, QP)
    nc.compile()
    _NC_CACHE[QP] = nc
    return nc


def run(inputs: dict, trace: bool = False):
    """Run on 8 cores; returns (full_output, BassKernelResults)."""
    in_maps, QP, order, cnt, bo_np = host_prepare(**inputs)
    nc = _get_program(QP)
    res = bass_utils.run_bass_kernel_spmd(
        nc, in_maps, core_ids=list(range(NCORES)), trace=trace,
    )
    out = np.empty((N, S, L, E), np.float32)
    out[:] = bo_np  # masked query rows: attention output is 0, fc adds bo
    for n in range(NCORES):
        # (S, P, 4, 2, QP) -> (S, E, QP): e = (jp*2 + j)*128 + p
        oT = np.asarray(res.results[n]["outT"]).astype(np.float32)
        oT = oT.transpose(0, 2, 3, 1, 4).reshape(S, E, QP)
        for s in range(S):
            c = int(cnt[n, s])
            if c:
                out[n, s, order[n, s, :c], :] = oT[s, :, :c].T + bo_np
    return out, res


def kernel(**inputs) -> np.ndarray:
    out, _ = run(inputs, trace=False)
    return out


# revision 25
# speedup vs baseline: 1.1094x; 1.0070x over previous
"""Trainium2 Bass kernel for nn_EntailmentSelfAttention (8-core data parallel).

Problem (per batch element n, sentence s):
  q/k/v head projections (shared per-head weights), energy = q @ k.T per head,
  query-position masking, softmax over the QUERY axis, out = attn @ v,
  fc_out: out @ Wo.T + bo.

Mapping (one (n) per NeuronCore; S=2 sentences iterated inside):
  - Tensors kept "transposed" on-chip: head-dim/embed-dim on partitions,
    sequence on the free axis, so the softmax (over queries) reduces along the
    free axis.
  - The V projection is folded into fc_out on the host:
      out = concat_h((attn_h @ xv_h) @ Wv.T) @ Wo.T = concat_h(attn_h @ xv_h) @ Wcomb
  - The query mask enters the energy matmul as a 65th contraction row:
      kT_ext row64 = 1, qT_ext row64 = -3000 * (1 - mask_q); after the 1/sqrt(L)
      softmax scale the exp underflows to 0 exactly, matching -1e20 semantics.
  - The K projection is folded into the q-side projection on the host:
      energy^T = xk^T (Wk^T Wq) xq.
  - Masked query positions are dropped on the host (compaction to QP columns);
    QP is padded to a multiple of 16.  Their output rows are exactly the fc
    bias; the bias for surviving rows is also added on the host.
  - Energy PSUM tiles are 2-bank pairs [128, 2, 512] so the exp activation,
    the PSUM->SBUF copies and the fc output copies each cover two heads /
    two fc column-tiles per instruction.
  - Softmax denominators: a tunable subset of (s,g,c) units uses the Exp
    activation's accum_out (scalar engine); the rest use a merged DVE
    tensor_reduce over the bf16 attention pair-tile.
  - The 1/rowsum normalization is folded into a per-k-row scale of xv before
    the attn @ xv matmul (tensor_tensor, optionally on gpsimd).
"""

import math

import numpy as np

import concourse.bass as bass
import concourse.tile as tile
from concourse import bacc, mybir
from concourse import bass_utils

# problem shapes (hardcoded per the harness contract)
N, S, L, E, H = 8, 2, 512, 1024, 16
D = E // H  # 64
DX = D + 1  # extended head dim (projection + mask/ones row)
P = 128
NCORES = 8
LC = L // P  # 4 l-chunks
GH = 4  # heads per group
NG = H // GH  # 4 groups
BMASK = 3000.0
QP_MIN = 256
SCALE = 1.0 / math.sqrt(float(L))

F32 = mybir.dt.float32
BF16 = mybir.dt.bfloat16

# rowsum strategy per (s, g, c) unit index (0..31):
#   "acc"    - per-head exp with scalar-engine fused accumulate
#   "reduce" - merged pair exp + DVE tensor_reduce per pair
#   "ts"     - merged pair exp + per-head DVE tensor_scalar w/ accum_out
#              (single-src op: eligible for DVE 2x/4x perf modes)
N_ACC = 0
ACC_UNITS = frozenset(
    u for u in range(32) if (u * N_ACC) // 32 != ((u + 1) * N_ACC) // 32
)
DVE_ROWSUM = "reduce"
# xvs scaling engine: "gpsimd" offloads the tensor_tensor to the Pool engine
XVS_ENGINE = "gpsimd"


def build_kernel_body(tc, outs, ins, QPs):
    nc = tc.nc
    QPX = max(QPs)

    def _c(ap):
        # sim path: run_kernel declares DRAM as plain fp32; view as bf16
        return ap if ap.dtype == BF16 else ap.bitcast(BF16)

    xq, xk, xv = _c(ins["xq"]), _c(ins["xk"]), _c(ins["xv"])
    wq, wcomb = _c(ins["wq"]), _c(ins["wcomb"])
    outT = outs["outT"]

    xvs_eng = nc.gpsimd if XVS_ENGINE == "gpsimd" else nc.vector

    import contextlib

    with contextlib.ExitStack() as ctx:
        ek = ctx.enter_context
        consts = ek(tc.tile_pool(name="consts", bufs=1))
        xqpool = ek(tc.tile_pool(name="xqp", bufs=5))
        xkpool = ek(tc.tile_pool(name="xkp", bufs=5))
        xvpool = ek(tc.tile_pool(name="xvp", bufs=2))
        qepool = ek(tc.tile_pool(name="qe", bufs=2))
        atpool = ek(tc.tile_pool(name="attn", bufs=4))
        scrpool = ek(tc.tile_pool(name="scrap", bufs=2))
        sumpool = ek(tc.tile_pool(name="sums", bufs=8))
        xvspool = ek(tc.tile_pool(name="xvs", bufs=4))
        ztpool = ek(tc.tile_pool(name="zt", bufs=2))
        otpool = ek(tc.tile_pool(name="out", bufs=3))
        pp_e = ek(tc.tile_pool(name="pp_e", bufs=3, space="PSUM"))
        pp_z = ek(tc.tile_pool(name="pp_z", bufs=1, space="PSUM"))

        wq_sb = consts.tile([DX, DX], BF16, tag="wq")
        wcomb_sb = consts.tile([P, E // P, E], BF16, tag="wcomb")

        # startup: group-0 inputs fan out across engine queues so the DGE
        # configs don't serialize; everything else trickles in on sync/gpsimd.
        xq_gs = {}
        xk_gs = {}
        xv_sbs = {}

        def load_group(s, g, qeng, keng):
            qp = QPs[s]
            t = xqpool.tile([DX, GH, qp], BF16, tag="xq_g", name=f"xq_{s}_{g}")
            qeng.dma_start(t[:], xq[s, g, :, :, :qp])
            xq_gs[(s, g)] = t
            t = xkpool.tile([DX, GH, L], BF16, tag="xk_g", name=f"xk_{s}_{g}")
            keng.dma_start(t[:], xk[s, g])
            xk_gs[(s, g)] = t

        nc.scalar.dma_start(wq_sb[:], wq[:])
        load_group(0, 0, nc.sync, nc.gpsimd)
        xv_sbs[0] = xvpool.tile([P, LC, E], BF16, tag="xv", name="xv_0")
        nc.gpsimd.dma_start(xv_sbs[0][:, 0, :], xv[0, 0])
        for c in range(1, LC):
            nc.gpsimd.dma_start(xv_sbs[0][:, c, :], xv[0, c])
        for g in range(1, NG):
            load_group(0, g, nc.sync, nc.sync)
        nc.gpsimd.dma_start(
            wcomb_sb[:], wcomb.rearrange("(eo p) j -> p eo j", p=P))

        ZTs = {}

        def emit_fc(s, jp):
            # fc_out: two column-tiles per PSUM pair-slot; bias added on host
            qp = QPs[s]
            fp = pp_e.tile([P, 2, 512], F32, tag="ep", name=f"fp_{s}_{jp}")
            for j in range(2):
                jt = 2 * jp + j
                for eo in range(E // P):
                    nc.tensor.matmul(
                        fp[:, j, :qp],
                        wcomb_sb[:, eo, jt * P:(jt + 1) * P],
                        ZTs[s][:, eo, :],
                        start=(eo == 0),
                        stop=(eo == E // P - 1))
            ot = otpool.tile([P, 2, qp], BF16, tag="ot", name=f"ot_{s}_{jp}")
            nc.vector.tensor_copy(ot[:], fp[:, :, :qp])
            nc.sync.dma_start(outT[s, :, jp, :, :qp], ot[:])

        def emit_av(zp, qp, xvs, at, c):
            for i in range(GH):
                nc.tensor.matmul(
                    zp[(i % 2) * D:(i % 2 + 1) * D, i // 2, :qp],
                    xvs[:, i],
                    at[:, i, :],
                    start=(c == 0),
                    stop=(c == LC - 1),
                    skip_group_check=True)

        def emit_qproj(s, g):
            # q projections: two heads per PSUM pair-slot; the two merged
            # copies split across scalar/vector so neither queue head-blocks
            qp = QPs[s]
            xq_g = xq_gs[(s, g)]
            qe = qepool.tile([DX, GH, qp], BF16, tag="qe", name=f"qe_{s}_{g}")
            for p2 in range(GH // 2):
                pq = pp_e.tile([P, 2, 512], F32, tag="ep", name="pq")
                for j in range(2):
                    nc.tensor.matmul(
                        pq[:DX, j, :qp], wq_sb[:], xq_g[:, 2 * p2 + j, :],
                        start=True, stop=True)
                if p2 == 0:
                    nc.scalar.copy(qe[:, :2, :], pq[:DX, :, :qp])
                else:
                    nc.vector.tensor_copy(qe[:, 2:, :], pq[:DX, :, :qp])
            return qe

        qe_next = None
        for s in range(S):
            qp = QPs[s]
            xv_sb = xv_sbs[s]
            ZTs[s] = ztpool.tile([P, E // P, qp], BF16, tag="zt", name=f"zt_{s}")
            for g in range(NG):
                xk_g = xk_gs[(s, g)]
                # stream in the next sentence's inputs two groups ahead
                if s == 0 and g >= 1 and g <= 2:
                    load_group(1, 2 * (g - 1), nc.sync, nc.sync)
                    load_group(1, 2 * (g - 1) + 1, nc.sync, nc.sync)
                if s == 0 and g == 3:
                    xv_sbs[1] = xvpool.tile([P, LC, E], BF16, tag="xv",
                                            name="xv_1")
                    for c in range(LC):
                        nc.sync.dma_start(xv_sbs[1][:, c, :], xv[1, c])

                qe = emit_qproj(s, g) if qe_next is None else qe_next
                qe_next = None

                zp = pp_z.tile([P, 2, 512], F32, tag="zp", name=f"zp_{s}_{g}")
                pend_av = None  # (xvs, at, c) for software-pipelined attn@v
                for c in range(LC):
                    unit = (s * NG + g) * LC + c
                    acc_mode = unit in ACC_UNITS
                    rsum = sumpool.tile([P, GH], F32, tag="rsum")
                    at = atpool.tile([P, GH, qp], BF16, tag="at", name="at")
                    for p2 in range(GH // 2):
                        ep = pp_e.tile([P, 2, 512], F32, tag="ep", name="ep")
                        for j in range(2):
                            i = 2 * p2 + j
                            nc.tensor.matmul(
                                ep[:, j, :qp],
                                xk_g[:, i, c * P:(c + 1) * P],
                                qe[:, i, :],
                                start=True, stop=True)
                        if acc_mode:
                            for j in range(2):
                                i = 2 * p2 + j
                                nc.scalar.activation(
                                    at[:, i, :], ep[:, j, :qp],
                                    mybir.ActivationFunctionType.Exp,
                                    scale=SCALE,
                                    accum_out=rsum[:, i:i + 1])
                        else:
                            nc.scalar.activation(
                                at[:, 2 * p2:2 * p2 + 2, :], ep[:, :, :qp],
                                mybir.ActivationFunctionType.Exp,
                                scale=SCALE)
                            nc.vector.tensor_reduce(
                                rsum[:, 2 * p2:2 * p2 + 2],
                                at[:, 2 * p2:2 * p2 + 2, :],
                                axis=mybir.AxisListType.X,
                                op=mybir.AluOpType.add)
                    if c == LC - 2:
                        # hoist the next group's q-projection: emitted early
                        # so its copies' deps are met when they reach the
                        # scalar/vector queue heads
                        if g < NG - 1:
                            qe_next = emit_qproj(s, g + 1)
                        elif s < S - 1:
                            qe_next = emit_qproj(s + 1, 0)
                    recip = sumpool.tile([P, GH], F32, tag="recip")
                    nc.vector.reciprocal(recip[:], rsum[:])
                    # xvs[p, i, d] = xv[p, c, (g*GH+i)*64+d] * recip[p, i]
                    xvs = xvspool.tile([P, GH, D], BF16, tag="xvs")
                    xvs_eng.tensor_tensor(
                        xvs[:],
                        xv_sb[:, c, g * GH * D:(g + 1) * GH * D].rearrange(
                            "p (h d) -> p h d", d=D),
                        recip[:, :, None].to_broadcast((P, GH, D)),
                        mybir.AluOpType.mult)
                    if pend_av is not None:
                        emit_av(zp, qp, *pend_av)
                    pend_av = (xvs, at, c)
                emit_av(zp, qp, *pend_av)
                nc.vector.tensor_copy(ZTs[s][:, 2 * g:2 * g + 2, :],
                                      zp[:, :, :qp])
                # interleave the previous sentence's fc into this attention
                if s == 1:
                    emit_fc(0, g)
            if s == 1:
                for jp in range(E // P // 2):
                    emit_fc(1, jp)


def host_prepare(values, keys, query, mask, Wv, Wk, Wq, Wo, bo):
    """Host-side sharding + layout + query compaction.

    The 16 (n, s) sentence-jobs are re-assigned across cores by surviving-
    query count: the 8 smallest-count jobs land in sentence-slot 0, the 8
    largest in slot 1, so slot 0 compiles with a smaller padded query count
    QP0 <= QP1.  Masked query positions are dropped entirely (their output
    is just bo); pad columns carry a -BMASK bias row so their exp is 0.
    """
    values = np.ascontiguousarray(np.asarray(values, dtype=np.float32))
    keys = np.asarray(keys, dtype=np.float32)
    query = np.asarray(query, dtype=np.float32)
    mask = np.asarray(mask)
    Wv = np.asarray(Wv, dtype=np.float32)
    Wk = np.asarray(Wk, dtype=np.float32)
    Wq = np.asarray(Wq, dtype=np.float32)
    Wo = np.asarray(Wo, dtype=np.float32)
    bo_np = np.ascontiguousarray(np.asarray(bo, dtype=np.float32))

    keep = mask[:, :, :, 0] != 0  # (N, S, L) True = query position survives
    cnt = keep.sum(-1)  # (N, S)
    # job assignment: sorted by count; slot 0 = N smallest, slot 1 = N largest
    flat = cnt.reshape(-1)
    order_jobs = np.argsort(flat, kind="stable")
    slot_jobs = [order_jobs[:N], order_jobs[N:]]

    def _qp(mx):
        q = int(np.ceil(max(int(mx), 1) / 16) * 16)
        return min(max(q, QP_MIN), L)

    QPs = tuple(_qp(flat[sj].max()) for sj in slot_jobs)
    QPX = max(QPs)
    # stable partition: surviving query indices first
    order = np.argsort(~keep, axis=-1, kind="stable")  # (N, S, L)

    qT = query.transpose(0, 1, 3, 2).reshape(N, S, H, D, L)
    kT = keys.transpose(0, 1, 3, 2).reshape(N, S, H, D, L)

    # gather+pad queries: (N, S, H, D, QPX)
    gidx = order[:, :, :QPX]  # (N, S, QPX)
    qTc = np.take_along_axis(
        qT, gidx[:, :, None, None, :].repeat(H, 2).repeat(D, 3), axis=4)
    pad = np.arange(QPX)[None, None, :] >= cnt[:, :, None]  # (N, S, QPX)
    qTc[pad[:, :, None, None, :].repeat(H, 2).repeat(D, 3)] = 0.0
    qb_row = np.where(pad, np.float32(-BMASK), np.float32(0.0)).astype(np.float32)
    xq = np.concatenate([qTc, qb_row[:, :, None, None, :].repeat(H, 2)], axis=3)
    # (N,S,H,DX,QPX) -> (N,S,NG,DX,GH,QPX) so each group is one contiguous DMA
    xq = np.ascontiguousarray(
        xq.reshape(N, S, NG, GH, DX, QPX).transpose(0, 1, 2, 4, 3, 5))

    ones_row = np.ones((N, S, H, 1, L), np.float32)
    xk = np.concatenate([kT, ones_row], axis=3)
    xk = np.ascontiguousarray(
        xk.reshape(N, S, NG, GH, DX, L).transpose(0, 1, 2, 4, 3, 5))

    # fused projection: energyT = xk^T (Wk^T Wq) xq -> yq = (Wk^T Wq) @ xqT,
    # lhsT[dj, di] = (Wk^T Wq)[di, dj] = (Wq^T Wk)[dj, di]
    wq_ext = np.zeros((DX, DX), np.float32)
    wq_ext[:D, :D] = Wq.T @ Wk
    wq_ext[D, D] = 1.0

    wcomb = np.zeros((E, E), np.float32)
    for h in range(H):
        wcomb[h * D:(h + 1) * D, :] = Wv.T @ Wo[:, h * D:(h + 1) * D].T
    wcomb = np.ascontiguousarray(wcomb)

    import ml_dtypes
    bf = ml_dtypes.bfloat16
    # values as (N, S, LC, P, E): per-(s, l-chunk) DMA granularity
    values_bf = values.reshape(N, S, LC, P, E).astype(bf)
    xq = xq.astype(bf)
    xk = xk.astype(bf)
    wq_ext = wq_ext.astype(bf)
    wcomb = np.ascontiguousarray(wcomb.astype(bf))
    shared = {"wq": wq_ext, "wcomb": wcomb}
    in_maps = []
    src_jobs = []  # per core: [(n, s) for slot 0, slot 1]
    for k in range(NCORES):
        jobs = [divmod(int(slot_jobs[sl][k]), S) for sl in range(S)]
        src_jobs.append(jobs)
        idx = ([j[0] for j in jobs], [j[1] for j in jobs])
        m = {
            "xq": np.ascontiguousarray(xq[idx]),
            "xk": np.ascontiguousarray(xk[idx]),
            "xv": np.ascontiguousarray(values_bf[idx]),
        }
        m.update(shared)
        in_maps.append(m)
    return in_maps, QPs, order, cnt, bo_np, src_jobs


_NC_CACHE = {}


def _get_program(QPs):
    nc = _NC_CACHE.get(QPs)
    if nc is not None:
        return nc
    QPX = max(QPs)
    nc = bacc.Bacc("TRN2", target_bir_lowering=False, debug=False,
                   num_devices=NCORES)
    ins = {
        "xq": nc.dram_tensor("xq", (S, NG, DX, GH, QPX), BF16, kind="ExternalInput").ap(),
        "xk": nc.dram_tensor("xk", (S, NG, DX, GH, L), BF16, kind="ExternalInput").ap(),
        "xv": nc.dram_tensor("xv", (S, LC, P, E), BF16, kind="ExternalInput").ap(),
        "wq": nc.dram_tensor("wq", (DX, DX), BF16, kind="ExternalInput").ap(),
        "wcomb": nc.dram_tensor("wcomb", (E, E), BF16, kind="ExternalInput").ap(),
    }
    outs = {
        "outT": nc.dram_tensor("outT", (S, P, E // P // 2, 2, QPX), BF16,
                               kind="ExternalOutput").ap(),
    }
    with tile.TileContext(nc) as tc:
        build_kernel_body(tc, outs, ins, QPs)
    nc.compile()
    _NC_CACHE[QPs] = nc
    return nc


def run(inputs: dict, trace: bool = False):
    """Run on 8 cores; returns (full_output, BassKernelResults)."""
    in_maps, QPs, order, cnt, bo_np, src_jobs = host_prepare(**inputs)
    nc = _get_program(QPs)
    res = bass_utils.run_bass_kernel_spmd(
        nc, in_maps, core_ids=list(range(NCORES)), trace=trace,
    )
    QPX = max(QPs)
    out = np.empty((N, S, L, E), np.float32)
    out[:] = bo_np  # masked query rows: attention output is 0, fc adds bo
    for k in range(NCORES):
        # (S, P, 4, 2, QPX) -> (S, E, QPX): e = (jp*2 + j)*128 + p
        oT = np.asarray(res.results[k]["outT"]).astype(np.float32)
        oT = oT.transpose(0, 2, 3, 1, 4).reshape(S, E, QPX)
        for sl in range(S):
            n, sj = src_jobs[k][sl]
            c = int(cnt[n, sj])
            if c:
                out[n, sj, order[n, sj, :c], :] = oT[sl, :, :c].T + bo_np
    return out, res


def kernel(**inputs) -> np.ndarray:
    out, _ = run(inputs, trace=False)
    return out
